# revision 4
# baseline (speedup 1.0000x reference)
"""Expert-parallel MoE MLP kernel for TRN2 (8 NeuronCores, 1 expert/core).

Math per core (expert e):
    h   = gelu(x_e @ w1_e + b1_e)      x_e: [4096, 1024], w1_e: [1024, 4096]
    out = h @ w2_e + b2_e              w2_e: [4096, 1024]

fp8 DoubleRow version: all matmuls run as float8e4 (e4m3) in DoubleRow
perf mode (2 k-planes per pass, 0.5 cycles/row -> 4x bf16 column rate).
Precision is recovered with a 3-term error-compensated decomposition:
    a @ b ~= a_hi@b_hi + a_lo@b_hi + a_hi@b_lo
where a_hi = e4m3(S*a), a_lo = e4m3(S*a - a_hi). This costs 3 fp8 terms
= 0.75x the bf16 PE time and lands ~0.27% end-to-end error (vs 0.34%
for the bf16 kernel). Scales: x*16, w1*64, w2*64; h unscaled. The mm1
psum carries 1024*(x@w1) and ACT applies gelu with scale 1/1024 (+b1);
the mm2 psum carries 64*(h@w2) and the drain applies *1/64 (+b2).

On device per chunk of 512 tokens:
  - mm1: 32 H-tiles, each a 12-matmul DoubleRow psum group (3 terms x 4
    D-pair planes); ACT gelu -> h bf16; Pool casts h->e4m3 (h_hi); DVE
    subtracts (h - h_hi) -> e4m3 (h_lo).
  - mm2: 8 psum groups of 48 DoubleRow matmuls (3 terms x 16 H-pairs);
    DVE scales by 1/64, Pool adds b2, DMA out.
Host-side prep (inside kernel(), part of the sharding step) builds the
hi/lo fp8 planes in the exact SBUF pair-plane layouts so every weight
DMA is a contiguous [128, 2048] tile copy.
"""

import numpy as np
import ml_dtypes

import concourse.bacc as bacc
import concourse.bass as bass
import concourse.mybir as mybir
import concourse.tile as tile
from concourse import bass_utils

P = 128
D = 1024
H = 4096
NTOK = 4096  # B*N per expert
NCORES = 8
CHUNK = 512  # tokens per pipeline chunk
NCHUNK = NTOK // CHUNK
F32 = mybir.dt.float32
BF16 = mybir.dt.bfloat16
E4 = mybir.dt.float8e4
GELU = mybir.ActivationFunctionType.Gelu
DR = mybir.MatmulPerfMode.DoubleRow

KP = D // (2 * P)   # 4   k-pair planes of D
HP = H // (2 * P)   # 16  k-pair planes of H
HM = H // P         # 32  H tiles (mm1 outputs)
HQ = 4              # w1 column quarters (1024 wide)
TSUB = CHUNK // P   # 4 token subtiles per chunk
DC = D // 512       # 2   512-wide output column chunks

SX = 16.0   # x pre-scale into e4m3
SW = 64.0   # w1/w2 pre-scale into e4m3
INV_PS1 = 1.0 / (SX * SW)   # gelu input scale
INV_PS2 = 1.0 / SW          # output scale


def build_program(act=GELU):
    nc = bacc.Bacc("TRN2", target_bir_lowering=False, debug=False,
                   num_devices=NCORES)

    # x hi/lo: [kp*128+p, j*4096+t]
    xh_d = nc.dram_tensor("xh", (KP * P, 2 * NTOK), E4,
                          kind="ExternalInput").ap()
    xl_d = nc.dram_tensor("xl", (KP * P, 2 * NTOK), E4,
                          kind="ExternalInput").ap()
    # w1 hi/lo: [(kp*4+q)*128+p, j*1024+c]
    w1h_d = nc.dram_tensor("w1h", (KP * HQ * P, 2 * D), E4,
                           kind="ExternalInput").ap()
    w1l_d = nc.dram_tensor("w1l", (KP * HQ * P, 2 * D), E4,
                           kind="ExternalInput").ap()
    # w2 hi/lo: [hp*128+p, j*1024+c]
    w2h_d = nc.dram_tensor("w2h", (HP * P, 2 * D), E4,
                           kind="ExternalInput").ap()
    w2l_d = nc.dram_tensor("w2l", (HP * P, 2 * D), E4,
                           kind="ExternalInput").ap()
    # biases pre-arranged on host: b1 as [128, 32] (H on partitions),
    # b2 replicated to [128, 1024]
    b1 = nc.dram_tensor("b1", (P, HM), F32, kind="ExternalInput").ap()
    b2 = nc.dram_tensor("b2", (P, D), F32, kind="ExternalInput").ap()
    out = nc.dram_tensor("out", (NTOK, D), F32, kind="ExternalOutput").ap()

    with tile.TileContext(nc) as tc:
        with (
            tc.tile_pool(name="consts", bufs=1) as consts,
            tc.tile_pool(name="weights", bufs=1) as wpool,
            tc.tile_pool(name="xt", bufs=2) as xtp,
            tc.tile_pool(name="ht", bufs=1) as htp,
            tc.tile_pool(name="hbf", bufs=3) as hbfp,
            tc.tile_pool(name="outp", bufs=2) as outp,
            tc.tile_pool(name="p1", bufs=4, space="PSUM") as p1p,
            tc.tile_pool(name="p2", bufs=4, space="PSUM") as p2p,
        ):
            def load_x_chunk(c, eng_hi, eng_lo):
                """x pair tiles [128, 2, 512] for chunk c (hi and lo)."""
                xh = [xtp.tile([P, 2, CHUNK], E4, tag=f"xh{kp}",
                               name=f"xh{kp}_c{c}") for kp in range(KP)]
                xl = [xtp.tile([P, 2, CHUNK], E4, tag=f"xl{kp}",
                               name=f"xl{kp}_c{c}") for kp in range(KP)]
                c0 = c * CHUNK
                for kp in range(KP):
                    for j in range(2):
                        eng_hi.dma_start(
                            xh[kp][:, j:j + 1, :],
                            xh_d[kp * P:(kp + 1) * P,
                                 j * NTOK + c0:j * NTOK + c0 + CHUNK])
                        eng_lo.dma_start(
                            xl[kp][:, j:j + 1, :],
                            xl_d[kp * P:(kp + 1) * P,
                                 j * NTOK + c0:j * NTOK + c0 + CHUNK])
                return xh, xl

            # warmup matmuls on a zeroed tile: the PE clock ramps with
            # continuous activity, so fill the initial DMA-wait window
            # with array work instead of idling at low p-state
            wz = consts.tile([P, CHUNK], BF16, tag="warmz")
            nc.vector.memset(wz, 0.0)
            wps = p1p.tile([P, CHUNK], F32, tag="p1", name="warm_ps")
            for _ in range(6):
                nc.tensor.matmul(wps, wz[:, 0:P], wz,
                                 start=True, stop=True,
                                 skip_group_check=True)

            # ---- weight + first-chunk DMA schedule ----
            # chunk-0 x_hi interleaved with the first w1_hi quarter
            # (per-plane halves) so mm1's first groups unblock after ~1.5MB
            w1t = {"h": [[None] * HQ for _ in range(KP)],
                   "l": [[None] * HQ for _ in range(KP)]}

            def w1_tile(hl, kp, q):
                t = wpool.tile([P, 2, D], E4, tag=f"w1{hl}_{kp}_{q}",
                               name=f"w1{hl}_{kp}_{q}")
                w1t[hl][kp][q] = t
                return t

            xh0 = [xtp.tile([P, 2, CHUNK], E4, tag=f"xh{kp}",
                            name=f"xh{kp}_c0") for kp in range(KP)]
            xl0 = [xtp.tile([P, 2, CHUNK], E4, tag=f"xl{kp}",
                            name=f"xl{kp}_c0") for kp in range(KP)]
            for kp in range(KP):
                for j in range(2):
                    nc.sync.dma_start(
                        xh0[kp][:, j:j + 1, :],
                        xh_d[kp * P:(kp + 1) * P, j * NTOK:j * NTOK + CHUNK])
                t = w1_tile("h", kp, 0)
                row0 = (kp * HQ) * P
                for j in range(2):
                    nc.gpsimd.dma_start(
                        t[:, j:j + 1, :],
                        w1h_d[row0:row0 + P, j * D:(j + 1) * D])
                if kp == 0:
                    # b1 rides early on sync — tiny, needed by the first
                    # gelu, must not trail the queue
                    b1_sb = consts.tile([P, HM], F32, tag="b1")
                    nc.sync.dma_start(b1_sb, b1)
            for kp in range(KP):
                for j in range(2):
                    nc.sync.dma_start(
                        xl0[kp][:, j:j + 1, :],
                        xl_d[kp * P:(kp + 1) * P, j * NTOK:j * NTOK + CHUNK])
                t = w1_tile("l", kp, 0)
                row0 = (kp * HQ) * P
                nc.gpsimd.dma_start(t, w1l_d[row0:row0 + P, :])

            # remaining w1 quarters, alternating DMA queues
            n = 0
            for q in range(1, HQ):
                for hl in ("h", "l"):
                    src = w1h_d if hl == "h" else w1l_d
                    for kp in range(KP):
                        row0 = (kp * HQ + q) * P
                        eng = nc.sync if n % 2 == 0 else nc.gpsimd
                        eng.dma_start(w1_tile(hl, kp, q),
                                      src[row0:row0 + P, :])
                        n += 1

            # w2 on the scalar queue (idle until mm1 compute starts)
            w2t = {"h": [], "l": []}
            for hp in range(HP):
                for hl, src in (("h", w2h_d), ("l", w2l_d)):
                    t = wpool.tile([P, 2, D], E4, tag=f"w2{hl}_{hp}",
                                   name=f"w2{hl}_{hp}")
                    w2t[hl].append(t)
                    nc.scalar.dma_start(t, src[hp * P:(hp + 1) * P, :])

            # b2 (replicated on host) — needed from the first out tile
            b2_rep = consts.tile([P, D], F32, tag="b2rep")
            nc.gpsimd.dma_start(b2_rep, b2)

            # ht pair tiles [128, 2, 512] (hi/lo), reused across chunks
            ht_hi = [htp.tile([P, 2, CHUNK], E4, tag=f"hthi{hp}",
                              name=f"hthi{hp}") for hp in range(HP)]
            ht_lo = [htp.tile([P, 2, CHUNK], E4, tag=f"htlo{hp}",
                              name=f"htlo{hp}") for hp in range(HP)]

            # ---- main pipeline over token chunks ----
            for c in range(NCHUNK):
                if c == 0:
                    xh, xl = xh0, xl0
                else:
                    xh, xl = xnext

                if c < NCHUNK - 1:
                    # prefetch next chunk during this chunk's mm1
                    xnext = load_x_chunk(c + 1, nc.sync, nc.gpsimd)

                # mm1 + gelu -> h_bf16; split into e4m3 hi/lo pair tiles
                for hm in range(HM):
                    q, hcol = hm // (HM // HQ), (hm % (HM // HQ)) * P
                    p1 = p1p.tile([P, CHUNK], F32, tag="p1",
                                  name=f"p1_c{c}h{hm}")
                    mms = [(xh, w1t["h"]), (xl, w1t["h"]), (xh, w1t["l"])]
                    for t, (xsrc, wsrc) in enumerate(mms):
                        for kp in range(KP):
                            nc.tensor.matmul(
                                p1,
                                wsrc[kp][q][:, :, hcol:hcol + P],
                                xsrc[kp],
                                start=(t == 0 and kp == 0),
                                stop=(t == 2 and kp == KP - 1),
                                perf_mode=DR)
                    hbf = hbfp.tile([P, CHUNK], BF16, tag="hbf",
                                    name=f"hbf_c{c}h{hm}")
                    nc.scalar.activation(hbf, p1, act,
                                         bias=b1_sb[:, hm:hm + 1],
                                         scale=INV_PS1)
                    hp, j = hm // 2, hm % 2
                    hi = ht_hi[hp][:, j:j + 1, :]
                    lo = ht_lo[hp][:, j:j + 1, :]
                    nc.gpsimd.tensor_copy(hi, hbf)
                    nc.vector.tensor_sub(lo, hbf, hi)

                # mm2 (+b2, *1/64) -> out
                for ts in range(TSUB):
                    last = (c == NCHUNK - 1 and ts == TSUB - 1)
                    p2s = [p2p.tile([P, 512], F32, tag="p2",
                                    name=f"p2_c{c}t{ts}d{dc}")
                           for dc in range(DC)]
                    tok = slice(ts * P, (ts + 1) * P)
                    for hp in range(HP):
                        mms = [(ht_hi, w2t["h"]), (ht_lo, w2t["h"]),
                               (ht_hi, w2t["l"])]
                        for t, (hsrc, wsrc) in enumerate(mms):
                            for dc in range(DC):
                                nc.tensor.matmul(
                                    p2s[dc],
                                    hsrc[hp][:, :, tok],
                                    wsrc[hp][:, :, dc * 512:(dc + 1) * 512],
                                    start=(hp == 0 and t == 0),
                                    stop=(hp == HP - 1 and t == 2),
                                    perf_mode=DR)
                    r0 = c * CHUNK + ts * P
                    if not last:
                        for dc in range(DC):
                            om = outp.tile([P, 512], F32, tag="otm",
                                           name=f"om_c{c}t{ts}d{dc}")
                            ot = outp.tile([P, 512], F32, tag="ot",
                                           name=f"ot_c{c}t{ts}d{dc}")
                            nc.vector.tensor_scalar_mul(om, p2s[dc], INV_PS2)
                            nc.gpsimd.tensor_add(
                                ot, om, b2_rep[:, dc * 512:(dc + 1) * 512])
                            if c == NCHUNK - 1:
                                # keep gpsimd free of late DMA so its
                                # end-of-kernel barrier wake isn't on the
                                # critical path
                                oeng = nc.sync if dc == 0 else nc.scalar
                            else:
                                oeng = nc.sync if dc == 0 else nc.gpsimd
                            oeng.dma_start(
                                out[r0:r0 + P, dc * 512:(dc + 1) * 512], ot)
                    else:
                        # final subtile: 4 parallel [128,256] pieces on the
                        # sync+scalar queues to minimize the drain tail
                        for qd in range(4):
                            cl = qd * 256
                            om = outp.tile([P, 256], F32, tag=f"omq{qd}",
                                           name=f"om_last{qd}", bufs=1)
                            ot = outp.tile([P, 256], F32, tag=f"otq{qd}",
                                           name=f"ot_last{qd}", bufs=1)
                            nc.vector.tensor_scalar_mul(
                                om, p2s[qd // 2][:, (qd % 2) * 256:
                                                 (qd % 2) * 256 + 256],
                                INV_PS2)
                            nc.gpsimd.tensor_add(ot, om,
                                                 b2_rep[:, cl:cl + 256])
                            eng = nc.sync if qd % 2 == 0 else nc.scalar
                            eng.dma_start(out[r0:r0 + P, cl:cl + 256], ot)

    nc.compile()
    return nc


_CACHE: dict = {}


def _program():
    if "nc" not in _CACHE:
        _CACHE["nc"] = build_program()
    return _CACHE["nc"]


_E4NP = ml_dtypes.float8_e4m3


def _split(a, s):
    """a*s as e4m3 hi + e4m3 residual lo (hi+lo ~= a*s to ~0.1%)."""
    hi = (a * s).astype(_E4NP)
    lo = (a * s - hi.astype(np.float32)).astype(_E4NP)
    return hi, lo


def _in_maps(x, w1, b1, w2, b2):
    x = np.asarray(x, dtype=np.float32)
    w1 = np.asarray(w1, dtype=np.float32)
    b1 = np.asarray(b1, dtype=np.float32)
    w2 = np.asarray(w2, dtype=np.float32)
    b2 = np.asarray(b2, dtype=np.float32)
    maps = []
    for e in range(NCORES):
        xt = x[:, e].reshape(NTOK, D).T  # [D, NTOK]
        xhi, xlo = _split(xt, SX)

        def xarr(a):
            # [kp*128+p, j*4096+t]
            return np.ascontiguousarray(
                a.reshape(KP, 2, P, NTOK).transpose(0, 2, 1, 3)
                .reshape(KP * P, 2 * NTOK))

        w1hi, w1lo = _split(w1[e], SW)

        def w1arr(a):
            # [(kp*4+q)*128+p, j*1024+c]
            return np.ascontiguousarray(
                a.reshape(KP, 2, P, HQ, D).transpose(0, 3, 2, 1, 4)
                .reshape(KP * HQ * P, 2 * D))

        w2hi, w2lo = _split(w2[e], SW)

        def w2arr(a):
            # [hp*128+p, j*1024+c]
            return np.ascontiguousarray(
                a.reshape(HP, 2, P, D).transpose(0, 2, 1, 3)
                .reshape(HP * P, 2 * D))

        maps.append({
            "xh": xarr(xhi), "xl": xarr(xlo),
            "w1h": w1arr(w1hi), "w1l": w1arr(w1lo),
            "w2h": w2arr(w2hi), "w2l": w2arr(w2lo),
            "b1": np.ascontiguousarray(b1[e].reshape(HM, P).T),
            "b2": np.ascontiguousarray(np.broadcast_to(b2[e], (P, D))),
        })
    return maps


def _install_ntff_hook_shim():
    """Provide antenv.axon_hooks if the image lacks it, wiring the NTFF
    profile hook straight to libaxon_pjrt.so (mirrors trn_agent_boot)."""
    import sys
    try:
        from antenv.axon_hooks import get_axon_ntff_profile_hook  # noqa: F401
        return
    except ImportError:
        pass
    import contextlib
    import ctypes
    import types

    import antenv

    hook = None
    so_path = "/opt/axon/libaxon_pjrt.so"
    try:
        lib = ctypes.CDLL(so_path)
        if hasattr(lib, "axon_start_nrt_profile"):
            lib.axon_start_nrt_profile.argtypes = [
                ctypes.POINTER(ctypes.c_int64), ctypes.c_size_t]
            lib.axon_start_nrt_profile.restype = ctypes.c_int64
            lib.axon_stop_nrt_profile.argtypes = [ctypes.c_char_p]
            lib.axon_stop_nrt_profile.restype = ctypes.c_int64

            @contextlib.contextmanager
            def _hook(output_dir, device_ids):
                import jax
                jax.devices()
                if device_ids:
                    ids = (ctypes.c_int64 * len(device_ids))(*device_ids)
                    rc = lib.axon_start_nrt_profile(ids, len(device_ids))
                else:
                    rc = lib.axon_start_nrt_profile(None, 0)
                if rc != 0:
                    raise RuntimeError(f"axon_start_nrt_profile rc={rc}")
                try:
                    yield
                finally:
                    n = lib.axon_stop_nrt_profile(str(output_dir).encode())
                    print(f"ntff profile: {n} file(s) -> {output_dir}")

            hook = _hook
    except OSError:
        pass

    mod = types.ModuleType("antenv.axon_hooks")
    mod._hook = hook
    mod.get_axon_ntff_profile_hook = lambda: mod._hook
    mod.set_axon_ntff_profile_hook = lambda h: setattr(mod, "_hook", h)
    sys.modules["antenv.axon_hooks"] = mod
    antenv.axon_hooks = mod


def run_spmd(x, w1, b1, w2, b2, trace=False):
    if trace:
        _install_ntff_hook_shim()
    nc = _program()
    res = bass_utils.run_bass_kernel_spmd(
        nc, _in_maps(x, w1, b1, w2, b2), core_ids=list(range(NCORES)),
        trace=trace)
    outs = [r["out"].reshape(4, 1024, D) for r in res.results]
    full = np.stack(outs, axis=1).astype(np.float32)  # [4, 8, 1024, 1024]
    return full, res


def kernel(x, w1, b1, w2, b2):
    full, _ = run_spmd(x, w1, b1, w2, b2)
    return full


# revision 5
# speedup vs baseline: 1.8334x; 1.8334x over previous
"""Expert-parallel MoE MLP kernel for TRN2 (8 NeuronCores, 1 expert/core).

Math per core (expert e):
    h   = gelu(x_e @ w1_e + b1_e)      x_e: [4096, 1024], w1_e: [1024, 4096]
    out = h @ w2_e + b2_e              w2_e: [4096, 1024]

Host-side prep (inside kernel(), part of the sharding step): x_e is
transposed to [D, tok] and cast to bf16, w1/w2 are cast to bf16. On
device the kernel is then pure matmul work:
  - mm1: stationary = w1 tile [128(D), 128(H)], moving = xT tile
    [128(D), 512(tok)] -> PSUM hT tile; ACT applies exact-erf GELU (+b1
    as per-partition bias) PSUM->SBUF bf16.
  - mm2: stationary = hT tile [128(H), 128(tok)], moving = w2 tile
    [128(H), 512(D)] -> PSUM out tile (fp32); DVE adds b2, DMA out.
Weights live in SBUF as 32+32 independent [128,1024] bf16 tiles loaded
over both DMA paths (HWDGE via sync + SWDGE via gpsimd) in an order that
lets chunk-0 mm1 start after ~2MB of DMA instead of the full preload.
The final token subtile is drained as 4 parallel [128,256] pieces on
the sync+scalar HWDGE queues so the post-last-matmul tail is minimal.
All matmuls accumulate fp32 in PSUM; bf16 only rounds the inputs.
"""

import numpy as np
import ml_dtypes

import concourse.bacc as bacc
import concourse.bass as bass
import concourse.mybir as mybir
import concourse.tile as tile
from concourse import bass_utils

P = 128
D = 1024
H = 4096
NTOK = 4096  # B*N per expert
NCORES = 8
CHUNK = 512  # tokens per pipeline chunk
NCHUNK = NTOK // CHUNK
F32 = mybir.dt.float32
BF16 = mybir.dt.bfloat16
GELU = mybir.ActivationFunctionType.Gelu

DK = D // P    # 8   k-tiles of D
HM = H // P    # 32  tiles of H
HQ = 4         # W1 loaded in 4 column quarters (1024 wide)
TSUB = CHUNK // P  # 4 token subtiles per chunk
DC = D // 512  # 2   512-wide output column chunks


def build_program(act=GELU):
    nc = bacc.Bacc("TRN2", target_bir_lowering=False, debug=False,
                   num_devices=NCORES)

    xt_d = nc.dram_tensor("xt", (D, NTOK), BF16, kind="ExternalInput").ap()
    w1 = nc.dram_tensor("w1", (D, H), BF16, kind="ExternalInput").ap()
    # biases arrive pre-arranged from the host: b1 as [128, 32] (H on
    # partitions), b2 replicated to [128, 1024] — plain contiguous DMAs
    b1 = nc.dram_tensor("b1", (P, HM), F32, kind="ExternalInput").ap()
    w2 = nc.dram_tensor("w2", (H, D), BF16, kind="ExternalInput").ap()
    b2 = nc.dram_tensor("b2", (P, D), F32, kind="ExternalInput").ap()
    out = nc.dram_tensor("out", (NTOK, D), F32, kind="ExternalOutput").ap()

    with tile.TileContext(nc) as tc:
        with (
            tc.tile_pool(name="consts", bufs=1) as consts,
            tc.tile_pool(name="weights", bufs=1) as wpool,
            tc.tile_pool(name="xt", bufs=2) as xtp,
            tc.tile_pool(name="ht", bufs=1) as htp,
            tc.tile_pool(name="outp", bufs=4) as outp,
            tc.tile_pool(name="p1", bufs=4, space="PSUM") as p1p,
            tc.tile_pool(name="p2", bufs=4, space="PSUM") as p2p,
        ):
            def load_xt_chunk(c):
                xt = [xtp.tile([P, CHUNK], BF16, tag=f"xt{dk}",
                               name=f"xt{dk}_c{c}") for dk in range(DK)]
                for dk in range(DK):
                    eng = nc.sync if dk % 2 == 0 else nc.gpsimd
                    eng.dma_start(
                        xt[dk], xt_d[dk * P:(dk + 1) * P,
                                     c * CHUNK:(c + 1) * CHUNK])
                return xt

            # warmup matmuls on a zeroed tile: the PE clock ramps with
            # continuous activity, so fill the initial DMA-wait window
            # with array work instead of idling at low p-state (6 MMs
            # end right as the first real tiles land, ~10.2us)
            wz = consts.tile([P, CHUNK], BF16, tag="warmz")
            nc.vector.memset(wz, 0.0)
            wps = p1p.tile([P, CHUNK], F32, tag="p1", name="warm_ps")
            for _ in range(6):
                nc.tensor.matmul(wps, wz[:, 0:P], wz,
                                 start=True, stop=True,
                                 skip_group_check=True)


            # chunk-0 activations interleaved dk-major with the first W1
            # column-quarter so mm1's k-loop unblocks progressively
            w1t = [[None] * HQ for _ in range(DK)]

            def alloc_w1(dk, hq):
                t = wpool.tile([P, D], BF16, tag=f"w1_{dk}_{hq}",
                               name=f"w1_{dk}_{hq}")
                w1t[dk][hq] = t
                return t

            # quarter 0 arrives as two [128,512] column halves so the
            # first mm1 groups need only 2MB (x0 + half-quarter) in
            # flight instead of 3MB before the PE can start real work
            w1q0 = [[wpool.tile([P, 512], BF16, tag=f"w1_{dk}_0{h}",
                                name=f"w1_{dk}_0{h}") for h in range(2)]
                    for dk in range(DK)]
            xt0 = [xtp.tile([P, CHUNK], BF16, tag=f"xt{dk}",
                            name=f"xt{dk}_c0") for dk in range(DK)]
            for dk in range(DK):
                e0 = nc.sync if dk % 2 == 0 else nc.gpsimd
                e1 = nc.gpsimd if dk % 2 == 0 else nc.sync
                e0.dma_start(xt0[dk], xt_d[dk * P:(dk + 1) * P, 0:CHUNK])
                e1.dma_start(w1q0[dk][0], w1[dk * P:(dk + 1) * P, 0:512])
                if dk == 0:
                    # b1 rides second on sync — tiny (16KB), needed by
                    # the first gelu (~14us), must not lead the queue
                    b1_sb = consts.tile([P, HM], F32, tag="b1")
                    nc.sync.dma_start(b1_sb, b1)
            for dk in range(DK):
                eng = nc.sync if dk % 2 == 0 else nc.gpsimd
                eng.dma_start(w1q0[dk][1], w1[dk * P:(dk + 1) * P, 512:1024])

            # remaining W1 quarters, alternating DMA queues
            for hq in range(1, HQ):
                for dk in range(DK):
                    eng = nc.sync if (hq * DK + dk) % 2 == 0 else nc.gpsimd
                    eng.dma_start(alloc_w1(dk, hq),
                                  w1[dk * P:(dk + 1) * P,
                                     hq * D:(hq + 1) * D])

            # b2 (replicated on host) — needed from the first out tile ~95us
            b2_rep = consts.tile([P, D], F32, tag="b2rep")
            nc.gpsimd.dma_start(b2_rep, b2)

            w2t = []
            for hk in range(HM):
                t = wpool.tile([P, D], BF16, tag=f"w2_{hk}",
                               name=f"w2_{hk}")
                w2t.append(t)
                eng = nc.sync if hk % 2 == 0 else nc.gpsimd
                eng.dma_start(t, w2[hk * P:(hk + 1) * P, :])

            # ---- main pipeline over token chunks ----
            for c in range(NCHUNK):
                # xT tiles straight from DRAM (bf16), double-buffered so
                # chunk c+1 prefetches during chunk c.
                xt = xt0 if c == 0 else load_xt_chunk(c)

                # mm1 + gelu -> hT tiles (bf16, H on partitions)
                ht = [htp.tile([P, CHUNK], BF16, tag=f"ht{hm}",
                               name=f"ht{hm}_c{c}") for hm in range(HM)]
                for hm in range(HM):
                    p1 = p1p.tile([P, CHUNK], F32, tag="p1",
                                  name=f"p1_c{c}h{hm}")
                    hq, hcol = hm // (HM // HQ), (hm % (HM // HQ)) * P
                    for dk in range(DK):
                        if hq == 0:
                            stat = w1q0[dk][hcol // 512][:, hcol % 512:
                                                         hcol % 512 + P]
                        else:
                            stat = w1t[dk][hq][:, hcol:hcol + P]
                        nc.tensor.matmul(
                            p1, stat, xt[dk],
                            start=(dk == 0), stop=(dk == DK - 1))
                    nc.scalar.activation(ht[hm], p1, act,
                                         bias=b1_sb[:, hm:hm + 1], scale=1.0)

                # mm2 (+b2) -> out
                for ts in range(TSUB):
                    last = (c == NCHUNK - 1 and ts == TSUB - 1)
                    p2s = [p2p.tile([P, 512], F32, tag="p2",
                                    name=f"p2_c{c}t{ts}d{dc}")
                           for dc in range(DC)]
                    for hk in range(HM):
                        lhsT = ht[hk][:, ts * P:(ts + 1) * P]
                        for dc in range(DC):
                            nc.tensor.matmul(
                                p2s[dc], lhsT,
                                w2t[hk][:, dc * 512:(dc + 1) * 512],
                                start=(hk == 0), stop=(hk == HM - 1))
                    r0 = c * CHUNK + ts * P
                    if not last:
                        for dc in range(DC):
                            ot = outp.tile([P, 512], F32, tag="ot",
                                           name=f"ot_c{c}t{ts}d{dc}")
                            nc.vector.tensor_add(
                                ot, p2s[dc],
                                b2_rep[:, dc * 512:(dc + 1) * 512])
                            if c == NCHUNK - 1:
                                # keep gpsimd free of late work so its
                                # end-of-kernel barrier wake isn't on
                                # the critical path
                                oeng = nc.sync if (ts + dc) % 2 == 0 \
                                    else nc.scalar
                            else:
                                oeng = nc.sync if (ts + dc) % 2 == 0 \
                                    else nc.gpsimd
                            oeng.dma_start(
                                out[r0:r0 + P, dc * 512:(dc + 1) * 512], ot)
                    else:
                        # final subtile: 4 parallel [128,256] pieces on
                        # the two HWDGE queues to minimize the drain
                        # tail after the last matmul
                        for q in range(4):
                            cl = q * 256
                            ot = outp.tile([P, 256], F32, tag=f"otq{q}",
                                           name=f"ot_last{q}")
                            nc.vector.tensor_add(
                                ot,
                                p2s[q // 2][:, (q % 2) * 256:
                                            (q % 2) * 256 + 256],
                                b2_rep[:, cl:cl + 256])
                            eng = nc.sync if q % 2 == 0 else nc.scalar
                            eng.dma_start(out[r0:r0 + P, cl:cl + 256], ot)

    nc.compile()
    return nc


_CACHE: dict = {}


def _program():
    if "nc" not in _CACHE:
        _CACHE["nc"] = build_program()
    return _CACHE["nc"]


def _in_maps(x, w1, b1, w2, b2):
    x = np.asarray(x, dtype=np.float32)
    w1 = np.asarray(w1, dtype=np.float32)
    b1 = np.asarray(b1, dtype=np.float32)
    w2 = np.asarray(w2, dtype=np.float32)
    b2 = np.asarray(b2, dtype=np.float32)
    bf = ml_dtypes.bfloat16
    maps = []
    for e in range(NCORES):
        xt_e = np.ascontiguousarray(
            x[:, e].reshape(NTOK, D).T.astype(bf))  # [D, NTOK] bf16
        maps.append({
            "xt": xt_e,
            "w1": np.ascontiguousarray(w1[e].astype(bf)),
            "b1": np.ascontiguousarray(b1[e].reshape(HM, P).T),
            "w2": np.ascontiguousarray(w2[e].astype(bf)),
            "b2": np.ascontiguousarray(
                np.broadcast_to(b2[e], (P, D))),
        })
    return maps


def _install_ntff_hook_shim():
    """Provide antenv.axon_hooks if the image lacks it, wiring the NTFF
    profile hook straight to libaxon_pjrt.so (mirrors trn_agent_boot)."""
    import sys
    try:
        from antenv.axon_hooks import get_axon_ntff_profile_hook  # noqa: F401
        return
    except ImportError:
        pass
    import contextlib
    import ctypes
    import types

    import antenv

    hook = None
    so_path = "/opt/axon/libaxon_pjrt.so"
    try:
        lib = ctypes.CDLL(so_path)
        if hasattr(lib, "axon_start_nrt_profile"):
            lib.axon_start_nrt_profile.argtypes = [
                ctypes.POINTER(ctypes.c_int64), ctypes.c_size_t]
            lib.axon_start_nrt_profile.restype = ctypes.c_int64
            lib.axon_stop_nrt_profile.argtypes = [ctypes.c_char_p]
            lib.axon_stop_nrt_profile.restype = ctypes.c_int64

            @contextlib.contextmanager
            def _hook(output_dir, device_ids):
                import jax
                jax.devices()
                if device_ids:
                    ids = (ctypes.c_int64 * len(device_ids))(*device_ids)
                    rc = lib.axon_start_nrt_profile(ids, len(device_ids))
                else:
                    rc = lib.axon_start_nrt_profile(None, 0)
                if rc != 0:
                    raise RuntimeError(f"axon_start_nrt_profile rc={rc}")
                try:
                    yield
                finally:
                    n = lib.axon_stop_nrt_profile(str(output_dir).encode())
                    print(f"ntff profile: {n} file(s) -> {output_dir}")

            hook = _hook
    except OSError:
        pass

    mod = types.ModuleType("antenv.axon_hooks")
    mod._hook = hook
    mod.get_axon_ntff_profile_hook = lambda: mod._hook
    mod.set_axon_ntff_profile_hook = lambda h: setattr(mod, "_hook", h)
    sys.modules["antenv.axon_hooks"] = mod
    antenv.axon_hooks = mod


def run_spmd(x, w1, b1, w2, b2, trace=False):
    if trace:
        _install_ntff_hook_shim()
    nc = _program()
    res = bass_utils.run_bass_kernel_spmd(
        nc, _in_maps(x, w1, b1, w2, b2), core_ids=list(range(NCORES)),
        trace=trace)
    outs = [r["out"].reshape(4, 1024, D) for r in res.results]
    full = np.stack(outs, axis=1).astype(np.float32)  # [4, 8, 1024, 1024]
    return full, res


def kernel(x, w1, b1, w2, b2):
    full, _ = run_spmd(x, w1, b1, w2, b2)
    return full
